# revision 1
# baseline (speedup 1.0000x reference)
"""Trainium2 Bass kernel for nn_CombinatorialClassifier (segment_reduce).

Strategy (8 NeuronCores, tensor-parallel over the num_partitionings axis):
  - Core i owns partitionings {2i, 2i+1}: a [2000, 2048] slice of W.
  - On device: logits = x @ Wshard.T (+ b row folded into the matmul),
    per-partitioning softmax -> probs [64, 2000], then a gpsimd ap_gather
    picks probs[b, idx(p, c)] for every class and the two partitionings are
    summed -> per-core partial [64, 50000].
  - Host: sum the 8 partials over cores (the all-reduce of the sharding
    hint), normalize over classes, log.

Gather layout: the probs tile is duplicated across SBUF partitions 0-63 /
64-127 so all 8 gpsimd Q7 cores work; quadrant A handles classes
[0, 25000), quadrant B [25000, 50000).
"""

import os
from contextlib import ExitStack

import numpy as np

import concourse.bacc as bacc
import concourse.mybir as mybir
import concourse.tile as tile
from concourse import bass_utils

B, P, K, C, D = 64, 16, 1000, 50000, 2048
ESP = 1e-20
NCORES = 8
PPC = P // NCORES        # partitionings per core
NLOC = PPC * K           # local logits width (2000)
NT = 500                 # matmul N-tile (PSUM bank: 500 fp32 <= 512)
NNT = NLOC // NT         # 4 N-tiles
DCH = D // 128           # 16 contraction chunks of 128
CQ = C // 2              # classes per gather quadrant stream (25000)
JC = 2048                # classes per gather call per quadrant

_chunks = []
_c = 0
while _c < CQ:
    _chunks.append(min(JC, CQ - _c))
    _c += JC
IDXCOLS = sum(2 * jc // 16 for jc in _chunks)  # int16 cols of gather indices

_F32 = mybir.dt.float32
_F16 = mybir.dt.float16
_I16 = mybir.dt.int16

_CACHE = {}
LAST_RESULTS = None


def _build_nc():
    nc = bacc.Bacc(
        "TRN2",
        target_bir_lowering=False,
        debug=False,
        enable_asserts=False,
        num_devices=NCORES,
    )
    xT_d = nc.dram_tensor("xT", [D, B], _F16, kind="ExternalInput")
    wtb_d = nc.dram_tensor("wtb", [D + 1, NLOC], _F16, kind="ExternalInput")
    idx_d = nc.dram_tensor("idx", [128, IDXCOLS], _I16, kind="ExternalInput")
    out_d = nc.dram_tensor("part_out", [B, C], _F32, kind="ExternalOutput")

    with tile.TileContext(nc) as tc, ExitStack() as ctx:
        const = ctx.enter_context(tc.tile_pool(name="const", bufs=1))
        wpool = ctx.enter_context(tc.tile_pool(name="w", bufs=3))
        spool = ctx.enter_context(tc.tile_pool(name="stats", bufs=1))
        gpool = ctx.enter_context(tc.tile_pool(name="g", bufs=3))
        apool = ctx.enter_context(tc.tile_pool(name="a", bufs=3))
        psum = ctx.enter_context(
            tc.tile_pool(name="psum", bufs=1, space="PSUM")
        )

        xt = const.tile([128, DCH, B], _F16)
        nc.sync.dma_start(xt[:], xT_d.ap().rearrange("(c p) b -> p c b", p=128))
        ones = const.tile([1, B], _F16)
        nc.vector.memset(ones[:], 1.0)
        bias = const.tile([1, NLOC], _F16)
        nc.sync.dma_start(bias[:], wtb_d[D : D + 1, :])
        idx_sb = const.tile([128, IDXCOLS], _I16)
        nc.sync.dma_start(idx_sb[:], idx_d.ap())
        gsrc = const.tile([128, NLOC], _F32)

        # ---- logits = x @ Wshard.T + b, accumulated in 4 PSUM banks ----
        ps = [psum.tile([B, NT], _F32, tag=f"ps{n}", name=f"ps{n}") for n in range(NNT)]
        for j in range(DCH):
            wt = wpool.tile([128, NLOC], _F16)
            nc.sync.dma_start(wt[:], wtb_d[128 * j : 128 * (j + 1), :])
            for n in range(NNT):
                nc.tensor.matmul(
                    ps[n][:],
                    xt[:, j, :],
                    wt[:, NT * n : NT * (n + 1)],
                    start=(j == 0),
                    stop=False,
                )
        for n in range(NNT):
            nc.tensor.matmul(
                ps[n][:],
                ones[:],
                bias[:, NT * n : NT * (n + 1)],
                start=False,
                stop=True,
            )

        # ---- per-partitioning softmax -> probs in gsrc[0:64] ----
        mx = spool.tile([B, NNT], _F32)
        for n in range(NNT):
            nc.vector.reduce_max(
                mx[:, n : n + 1], ps[n][:], axis=mybir.AxisListType.X
            )
        neg = spool.tile([B, PPC], _F32)
        for h in range(PPC):
            nc.vector.tensor_tensor(
                neg[:, h : h + 1],
                mx[:, 2 * h : 2 * h + 1],
                mx[:, 2 * h + 1 : 2 * h + 2],
                op=mybir.AluOpType.max,
            )
        nc.vector.tensor_scalar_mul(neg[:], neg[:], -1.0)
        sacc = spool.tile([B, NNT], _F32)
        for n in range(NNT):
            h = n // 2
            nc.scalar.activation(
                gsrc[0:B, NT * n : NT * (n + 1)],
                ps[n][:],
                mybir.ActivationFunctionType.Exp,
                bias=neg[:, h : h + 1],
                accum_out=sacc[:, n : n + 1],
            )
        rec = spool.tile([B, PPC], _F32)
        for h in range(PPC):
            nc.vector.tensor_tensor(
                rec[:, h : h + 1],
                sacc[:, 2 * h : 2 * h + 1],
                sacc[:, 2 * h + 1 : 2 * h + 2],
                op=mybir.AluOpType.add,
            )
        nc.vector.reciprocal(rec[:], rec[:])
        for h in range(PPC):
            nc.vector.tensor_scalar_mul(
                gsrc[0:B, K * h : K * (h + 1)],
                gsrc[0:B, K * h : K * (h + 1)],
                rec[:, h : h + 1],
            )
        # duplicate probs for the second gpsimd quadrant
        nc.sync.dma_start(gsrc[B : 2 * B, :], gsrc[0:B, :])

        # ---- gather + partitioning-sum + store ----
        offs = 0
        cum = 0
        for jc in _chunks:
            S = 2 * jc // 16
            g = gpool.tile([128, 2 * JC], _F32, tag="g")
            nc.gpsimd.ap_gather(
                g[:, 0 : 2 * jc],
                gsrc[:, :],
                idx_sb[:, offs : offs + S],
                channels=128,
                num_elems=NLOC,
                d=1,
                num_idxs=2 * jc,
            )
            a = apool.tile([128, JC], _F32, tag="a")
            nc.vector.tensor_add(a[:, 0:jc], g[:, 0:jc], g[:, jc : 2 * jc])
            nc.sync.dma_start(out_d[:, cum : cum + jc], a[0:B, 0:jc])
            nc.sync.dma_start(
                out_d[:, CQ + cum : CQ + cum + jc], a[B : 2 * B, 0:jc]
            )
            offs += S
            cum += jc

    nc.compile()
    return nc


def _host_inputs(x, W, b, part):
    """Per-core input maps: xT, wtb (W.T shard + bias row), gather indices."""
    xT = np.ascontiguousarray(x.T.astype(np.float16))
    part = np.asarray(part).astype(np.int64, copy=False)
    in_maps = []
    for i in range(NCORES):
        r0 = NLOC * i
        wtb = np.empty((D + 1, NLOC), np.float16)
        wtb[:D] = W[r0 : r0 + NLOC].T
        wtb[D] = b[r0 : r0 + NLOC]

        pa = part[2 * i] - (2 * i) * K            # [C] in [0, K)
        pb = part[2 * i + 1] - (2 * i + 1) * K + K  # [C] in [K, 2K)
        idxh = np.empty((128, IDXCOLS), np.int16)
        off = 0
        c0 = 0
        for jc in _chunks:
            S = 2 * jc // 16
            for q in range(2):
                base = q * CQ + c0
                L = np.concatenate(
                    [pa[base : base + jc], pb[base : base + jc]]
                ).astype(np.int16)
                blk = L.reshape(S, 16).T  # out col i <- (partition i%16, col i//16)
                for g4 in range(4):
                    p0 = q * 64 + g4 * 16
                    idxh[p0 : p0 + 16, off : off + S] = blk
            off += S
            c0 += jc
        in_maps.append({"xT": xT, "wtb": wtb, "idx": idxh})
    return in_maps


def kernel(**inputs):
    global LAST_RESULTS
    x = np.asarray(inputs["input"], dtype=np.float32)
    W = np.asarray(inputs["W"], dtype=np.float32)
    b = np.asarray(inputs["b"], dtype=np.float32)
    part = np.asarray(inputs["partitionings"])
    assert x.shape == (B, D) and W.shape == (P * K, D)

    if "nc" not in _CACHE:
        _CACHE["nc"] = _build_nc()
    nc = _CACHE["nc"]

    in_maps = _host_inputs(x, W, b, part)
    trace = bool(int(os.environ.get("BASSK_TRACE", "0")))
    res = bass_utils.run_bass_kernel_spmd(
        nc,
        in_maps,
        core_ids=list(range(NCORES)),
        trace=trace,
        tmpdir=os.environ.get("BASSK_TRACE_DIR") or None,
    )
    LAST_RESULTS = res

    acc = res.results[0]["part_out"].astype(np.float32)
    for i in range(1, NCORES):
        acc = acc + res.results[i]["part_out"]
    tot = acc.sum(axis=1, keepdims=True)
    return np.log(acc / tot + ESP).astype(np.float32)



# revision 4
# speedup vs baseline: 10.1540x; 10.1540x over previous
"""Trainium2 Bass kernel for nn_CombinatorialClassifier (segment_reduce).

Strategy (8 NeuronCores, tensor-parallel over the num_partitionings axis):
  Core i owns partitionings {2i, 2i+1} (a [2000, 2048] slice of W).
  On device:
    1. logits = x @ Wshard.T + b  (PE, fp16)
    2. per-partitioning softmax -> probs [64, 2048] fp16 (padded 1024/stream)
    3. probs transposed via PE -> probsT [128, 16, 64] (row k at partition
       k%128, chunk k//128; stream s rows at 1024s+k)
    4. "gather" as staircase one-hot matmuls: host sorts classes of each
       partitioning by partition idx k; each 32-row block of probsT is
       multiplied with a one-hot fp8 block S [32, NP] whose column j selects
       row k of the block -> PSUM [64, NP] = probs gathered in sorted order.
       Streams a/b go to PSUM partitions 0-63 / 64-127 (PE column tiling),
       evacuated to fp16 and DMA'd to DRAM in padded sorted order.
  Host: un-permute the 16 padded sorted streams, sum, normalize, log.

The Q7 ap_gather of the previous version (27 ns/idx, 1.43 ms total) is
replaced by PE matmuls at ~0.5 ns/gathered element.
"""

import os
from contextlib import ExitStack

import numpy as np

import concourse.bacc as bacc
import concourse.mybir as mybir
import concourse.tile as tile
from concourse import bass_utils

B, P, K, C, D = 64, 16, 1000, 50000, 2048
ESP = 1e-20
NCORES = 8
PPC = P // NCORES        # partitionings per core (2)
NLOC = PPC * K           # local logits width (2000)
NT = 500                 # logits matmul N-tile (PSUM bank: 500 fp32 <= 512)
NNT = NLOC // NT         # 4 N-tiles
DCH = D // 128           # 16 contraction chunks of 128
KP = 1024                # padded rows per stream (k space)
NBLK = KP // 32          # 32-row blocks per stream
NP = 1792                # padded sorted-class columns per block
NPAD = NBLK * NP         # padded sorted stream length (57344)

_F32 = mybir.dt.float32
_F16 = mybir.dt.float16
_F8 = mybir.dt.float8e4
_F8NP = mybir.dt.np(_F8)

_CACHE = {}
LAST_RESULTS = None


def _build_nc():
    nc = bacc.Bacc(
        "TRN2",
        target_bir_lowering=False,
        debug=False,
        enable_asserts=False,
        num_devices=NCORES,
    )
    xT_d = nc.dram_tensor("xT", [D, B], _F16, kind="ExternalInput")
    wtb_d = nc.dram_tensor("wtb", [D + 1, NLOC], _F16, kind="ExternalInput")
    s_d = nc.dram_tensor("sel", [128, 16, NP], _F8, kind="ExternalInput")
    id_d = nc.dram_tensor("id64", [B, B], _F16, kind="ExternalInput")
    out_d = nc.dram_tensor("pout", [PPC, B, NPAD], _F16, kind="ExternalOutput")
    probs_d = nc.dram_tensor("probs16", [B, 2 * KP], _F16, kind="ExternalOutput")

    with tile.TileContext(nc) as tc, ExitStack() as ctx:
        const = ctx.enter_context(tc.tile_pool(name="const", bufs=1))
        wpool = ctx.enter_context(tc.tile_pool(name="w", bufs=3))
        spool = ctx.enter_context(tc.tile_pool(name="stats", bufs=1))
        opool = ctx.enter_context(tc.tile_pool(name="o", bufs=3))

        xt = const.tile([128, DCH, B], _F16)
        nc.sync.dma_start(xt[:], xT_d.ap().rearrange("(c p) b -> p c b", p=128))
        ones = const.tile([1, B], _F16)
        nc.vector.memset(ones[:], 1.0)
        bias = const.tile([1, NLOC], _F16)
        nc.sync.dma_start(bias[:], wtb_d[D : D + 1, :])
        id64 = const.tile([B, B], _F16)
        nc.sync.dma_start(id64[:], id_d.ap())
        sel = const.tile([128, 16, NP], _F8)
        nc.sync.dma_start(sel[:], s_d.ap())

        gsrc = const.tile([B, 2 * KP], _F32)
        probs16 = const.tile([B, 2 * KP], _F16)
        nc.vector.memset(probs16[:], 0.0)
        probsT = const.tile([128, 16, B], _F16)

        # ---- phase 1: logits = x @ Wshard.T + b, softmax -> probs16 ----
        with tc.tile_pool(name="psum1", bufs=1, space="PSUM") as psum1:
            ps = [
                psum1.tile([B, NT], _F32, tag=f"ps{n}", name=f"ps{n}")
                for n in range(NNT)
            ]
            for j in range(DCH):
                wt = wpool.tile([128, NLOC], _F16)
                nc.sync.dma_start(wt[:], wtb_d[128 * j : 128 * (j + 1), :])
                for n in range(NNT):
                    nc.tensor.matmul(
                        ps[n][:],
                        xt[:, j, :],
                        wt[:, NT * n : NT * (n + 1)],
                        start=(j == 0),
                        stop=False,
                    )
            for n in range(NNT):
                nc.tensor.matmul(
                    ps[n][:],
                    ones[:],
                    bias[:, NT * n : NT * (n + 1)],
                    start=False,
                    stop=True,
                )

            mx = spool.tile([B, NNT], _F32)
            for n in range(NNT):
                nc.vector.reduce_max(
                    mx[:, n : n + 1], ps[n][:], axis=mybir.AxisListType.X
                )
            neg = spool.tile([B, PPC], _F32)
            for h in range(PPC):
                nc.vector.tensor_tensor(
                    neg[:, h : h + 1],
                    mx[:, 2 * h : 2 * h + 1],
                    mx[:, 2 * h + 1 : 2 * h + 2],
                    op=mybir.AluOpType.max,
                )
            nc.vector.tensor_scalar_mul(neg[:], neg[:], -1.0)
            sacc = spool.tile([B, NNT], _F32)
            # bank n holds logits cols [500n, 500n+500) of the k-contiguous
            # [0, 2000) space; padded target col = 1024h + k
            goff = [0, 500, KP, KP + 500]
            for n in range(NNT):
                h = n // 2
                nc.scalar.activation(
                    gsrc[0:B, goff[n] : goff[n] + NT],
                    ps[n][:],
                    mybir.ActivationFunctionType.Exp,
                    bias=neg[:, h : h + 1],
                    accum_out=sacc[:, n : n + 1],
                )
        rec = spool.tile([B, PPC], _F32)
        for h in range(PPC):
            nc.vector.tensor_tensor(
                rec[:, h : h + 1],
                sacc[:, 2 * h : 2 * h + 1],
                sacc[:, 2 * h + 1 : 2 * h + 2],
                op=mybir.AluOpType.add,
            )
        nc.vector.reciprocal(rec[:], rec[:])
        for h in range(PPC):
            nc.vector.tensor_scalar_mul(
                probs16[0:B, KP * h : KP * h + K],
                gsrc[0:B, KP * h : KP * h + K],
                rec[:, h : h + 1],
            )
        nc.sync.dma_start(probs_d.ap(), probs16[:, :])

        psum2 = ctx.enter_context(
            tc.tile_pool(name="psum2", bufs=3, space="PSUM")
        )
        psumt = ctx.enter_context(
            tc.tile_pool(name="psumt", bufs=2, space="PSUM")
        )

        # ---- phase 1.5: transpose probs16 -> probsT [128, 16, 64] ----
        for c in range(16):
            tp = psumt.tile([128, B], _F16, tag="tp")
            nc.tensor.transpose(
                tp[:, :], probs16[:, 128 * c : 128 * (c + 1)], id64[:, :]
            )
            nc.scalar.activation(
                probsT[:, c, :], tp[:, :], mybir.ActivationFunctionType.Copy
            )

        # ---- phase 2: staircase one-hot gather matmuls + evac + store ----
        for t in range(NBLK):
            prow = 32 * (t % 4)
            chunk = t // 4
            ot = opool.tile([128, NP], _F16, tag="ot")
            for j in range(4):
                w = 512 if j < 3 else NP - 3 * 512
                pst = psum2.tile([128, 512], _F32, tag="ps")
                for strm in range(PPC):
                    nc.tensor.matmul(
                        pst[B * strm : B * strm + B, 0:w],
                        probsT[prow : prow + 32, 8 * strm + chunk, :],
                        sel[
                            prow : prow + 32,
                            8 * strm + chunk,
                            512 * j : 512 * j + w,
                        ],
                        start=True,
                        stop=True,
                        tile_position=(prow, B * strm),
                    )
                if (t * 4 + j) % 2 == 0:
                    nc.vector.tensor_copy(
                        ot[:, 512 * j : 512 * j + w], pst[:, 0:w]
                    )
                else:
                    nc.scalar.activation(
                        ot[:, 512 * j : 512 * j + w],
                        pst[:, 0:w],
                        mybir.ActivationFunctionType.Copy,
                    )
            nc.sync.dma_start(
                out_d.ap()
                .rearrange("s b np -> (s b) np")[:, NP * t : NP * (t + 1)],
                ot[:, :],
            )

    nc.compile()
    return nc


def _host_inputs(x, W, b, part):
    """Per-core inputs + per-(core,stream) position maps for host unpermute."""
    xT = np.ascontiguousarray(x.T.astype(np.float16))
    id64 = np.eye(B, dtype=np.float16)
    part = np.asarray(part).astype(np.int64, copy=False)
    in_maps = []
    pos_maps = []       # [core][strm] -> int32 [C] padded position or -1
    overflows = []      # [core][strm] -> list of (class, k) fallen out of NP
    for i in range(NCORES):
        r0 = NLOC * i
        wtb = np.empty((D + 1, NLOC), np.float16)
        wtb[:D] = W[r0 : r0 + NLOC].T
        wtb[D] = b[r0 : r0 + NLOC]

        sel = np.zeros((128, 16, NP), _F8NP)
        pm_core = []
        ov_core = []
        for s in range(PPC):
            kloc = part[PPC * i + s] - (PPC * i + s) * K  # [C] in [0, K)
            order = np.argsort(kloc, kind="stable")
            k_sorted = kloc[order]
            blk = k_sorted >> 5                            # 32-row block id
            # rank within block
            starts = np.searchsorted(blk, np.arange(NBLK))
            rank = np.arange(C, dtype=np.int64) - starts[blk]
            ok = rank < NP
            prow = 32 * (blk % 4) + (k_sorted & 31)
            dim1 = 8 * s + (blk >> 2)
            sel[prow[ok], dim1[ok], rank[ok]] = 1.0
            pos = np.where(ok, blk * NP + rank, -1).astype(np.int64)
            pm = np.empty(C, np.int64)
            pm[order] = pos
            pm_core.append(pm)
            if not ok.all():
                bad = order[~ok]
                ov_core.append([(int(c), int(kloc[c])) for c in bad])
            else:
                ov_core.append([])
        pos_maps.append(pm_core)
        overflows.append(ov_core)
        in_maps.append({"xT": xT, "wtb": wtb, "sel": sel, "id64": id64})
    return in_maps, pos_maps, overflows


def kernel(**inputs):
    global LAST_RESULTS
    x = np.asarray(inputs["input"], dtype=np.float32)
    W = np.asarray(inputs["W"], dtype=np.float32)
    b = np.asarray(inputs["b"], dtype=np.float32)
    part = np.asarray(inputs["partitionings"])
    assert x.shape == (B, D) and W.shape == (P * K, D)

    if "nc" not in _CACHE:
        _CACHE["nc"] = _build_nc()
    nc = _CACHE["nc"]

    in_maps, pos_maps, overflows = _host_inputs(x, W, b, part)
    trace = bool(int(os.environ.get("BASSK_TRACE", "0")))
    res = bass_utils.run_bass_kernel_spmd(
        nc,
        in_maps,
        core_ids=list(range(NCORES)),
        trace=trace,
        tmpdir=os.environ.get("BASSK_TRACE_DIR") or None,
    )
    LAST_RESULTS = res

    acc = np.zeros((B, C), np.float32)
    for i in range(NCORES):
        pout = res.results[i]["pout"]          # [PPC, B, NPAD] fp16
        for s in range(PPC):
            acc += pout[s][:, pos_maps[i][s]].astype(np.float32)
            if overflows[i][s]:
                pr = res.results[i]["probs16"]  # [B, 2*KP] fp16
                for c, k in overflows[i][s]:
                    acc[:, c] += pr[:, KP * s + k].astype(np.float32)
    tot = acc.sum(axis=1, keepdims=True)
    return np.log(acc / tot + ESP).astype(np.float32)


# revision 10
# speedup vs baseline: 10.2862x; 1.0130x over previous
"""Trainium2 Bass kernel for nn_CombinatorialClassifier (segment_reduce).

Strategy (8 NeuronCores, tensor-parallel over the num_partitionings axis):
  Core i owns partitionings {2i, 2i+1} (a [2000, 2048] slice of W).
  On device:
    1. logits = x @ Wshard.T + b  (PE, fp16)
    2. per-partitioning softmax -> probs [64, 2048] fp16 (padded 1024/stream)
    3. probs transposed via PE -> probsT [128, 16, 64] (row k at partition
       k%128, chunk k//128; stream s rows at 1024s+k)
    4. "gather" as staircase one-hot matmuls: host sorts classes of each
       partitioning by partition idx k; each 32-row block of probsT is
       multiplied with a one-hot fp8 block S [32, NP] whose column j selects
       row k of the block -> PSUM [64, NP] = probs gathered in sorted order.
       Streams a/b go to PSUM partitions 0-63 / 64-127 (PE column tiling),
       evacuated to fp16 and DMA'd to DRAM in padded sorted order.
  Host: un-permute the 16 padded sorted streams, sum, normalize, log.

The Q7 ap_gather of the previous version (27 ns/idx, 1.43 ms total) is
replaced by PE matmuls at ~0.5 ns/gathered element.
"""

import os
from contextlib import ExitStack

import numpy as np

import concourse.bacc as bacc
import concourse.mybir as mybir
import concourse.tile as tile
from concourse import bass_utils

B, P, K, C, D = 64, 16, 1000, 50000, 2048
ESP = 1e-20
NCORES = 8
PPC = P // NCORES        # partitionings per core (2)
NLOC = PPC * K           # local logits width (2000)
NT = 500                 # logits matmul N-tile (PSUM bank: 500 fp32 <= 512)
NNT = NLOC // NT         # 4 N-tiles
DCH = D // 128           # 16 contraction chunks of 128
KP = 1024                # padded rows per stream (k space)
NBLK = KP // 32          # 32-row blocks per stream
NP = 1792                # padded sorted-class columns per block
NPAD = NBLK * NP         # padded sorted stream length (57344)

_F32 = mybir.dt.float32
_F16 = mybir.dt.float16
_F8 = mybir.dt.float8e4
_F8NP = mybir.dt.np(_F8)

_CACHE = {}
LAST_RESULTS = None


def _build_nc():
    nc = bacc.Bacc(
        "TRN2",
        target_bir_lowering=False,
        debug=False,
        enable_asserts=False,
        num_devices=NCORES,
    )
    xT_d = nc.dram_tensor("xT", [D, B], _F16, kind="ExternalInput")
    wtb_d = nc.dram_tensor("wtb", [D + 1, NLOC], _F16, kind="ExternalInput")
    s_d = nc.dram_tensor("sel", [128, 16, NP], _F8, kind="ExternalInput")
    id_d = nc.dram_tensor("id64", [B, B], _F16, kind="ExternalInput")
    out_d = nc.dram_tensor("pout", [PPC, B, NPAD], _F8, kind="ExternalOutput")
    probs_d = nc.dram_tensor("probs16", [B, 2 * KP], _F16, kind="ExternalOutput")

    with tile.TileContext(nc) as tc, ExitStack() as ctx:
        const = ctx.enter_context(tc.tile_pool(name="const", bufs=1))
        wpool = ctx.enter_context(tc.tile_pool(name="w", bufs=3))
        spool = ctx.enter_context(tc.tile_pool(name="stats", bufs=1))
        opool = ctx.enter_context(tc.tile_pool(name="o", bufs=3))

        xt = const.tile([128, DCH, B], _F16)
        nc.sync.dma_start(xt[:], xT_d.ap().rearrange("(c p) b -> p c b", p=128))
        ones = const.tile([1, B], _F16)
        nc.vector.memset(ones[:], 1.0)
        bias = const.tile([1, NLOC], _F16)
        nc.sync.dma_start(bias[:], wtb_d[D : D + 1, :])
        id64 = const.tile([B, B], _F16)
        nc.sync.dma_start(id64[:], id_d.ap())
        sel = const.tile([128, 16, NP], _F8)

        gsrc = const.tile([B, 2 * KP], _F32)
        probs16 = const.tile([B, 2 * KP], _F16)
        nc.vector.memset(probs16[:], 0.0)
        probsT = const.tile([128, 16, B], _F16)

        # ---- phase 1: logits = x @ Wshard.T + b, softmax -> probs16 ----
        with tc.tile_pool(name="psum1", bufs=1, space="PSUM") as psum1:
            ps = [
                psum1.tile([B, NT], _F32, tag=f"ps{n}", name=f"ps{n}")
                for n in range(NNT)
            ]
            for j in range(DCH):
                wt = wpool.tile([128, NLOC], _F16)
                nc.sync.dma_start(wt[:], wtb_d[128 * j : 128 * (j + 1), :])
                for n in range(NNT):
                    nc.tensor.matmul(
                        ps[n][:],
                        xt[:, j, :],
                        wt[:, NT * n : NT * (n + 1)],
                        start=(j == 0),
                        stop=False,
                    )
            # sel is only needed in phase 2 — issue its DMA after the W
            # chunks so it doesn't delay the logits pipeline
            nc.sync.dma_start(sel[:], s_d.ap())
            for n in range(NNT):
                nc.tensor.matmul(
                    ps[n][:],
                    ones[:],
                    bias[:, NT * n : NT * (n + 1)],
                    start=False,
                    stop=True,
                )

            mx = spool.tile([B, NNT], _F32)
            for n in range(NNT):
                nc.vector.reduce_max(
                    mx[:, n : n + 1], ps[n][:], axis=mybir.AxisListType.X
                )
            neg = spool.tile([B, PPC], _F32)
            for h in range(PPC):
                nc.vector.tensor_tensor(
                    neg[:, h : h + 1],
                    mx[:, 2 * h : 2 * h + 1],
                    mx[:, 2 * h + 1 : 2 * h + 2],
                    op=mybir.AluOpType.max,
                )
            nc.vector.tensor_scalar_mul(neg[:], neg[:], -1.0)
            sacc = spool.tile([B, NNT], _F32)
            # bank n holds logits cols [500n, 500n+500) of the k-contiguous
            # [0, 2000) space; padded target col = 1024h + k
            goff = [0, 500, KP, KP + 500]
            for n in range(NNT):
                h = n // 2
                nc.scalar.activation(
                    gsrc[0:B, goff[n] : goff[n] + NT],
                    ps[n][:],
                    mybir.ActivationFunctionType.Exp,
                    bias=neg[:, h : h + 1],
                    accum_out=sacc[:, n : n + 1],
                )
        rec = spool.tile([B, PPC], _F32)
        for h in range(PPC):
            nc.vector.tensor_tensor(
                rec[:, h : h + 1],
                sacc[:, 2 * h : 2 * h + 1],
                sacc[:, 2 * h + 1 : 2 * h + 2],
                op=mybir.AluOpType.add,
            )
        nc.vector.reciprocal(rec[:], rec[:])
        for h in range(PPC):
            nc.vector.tensor_scalar_mul(
                probs16[0:B, KP * h : KP * h + K],
                gsrc[0:B, KP * h : KP * h + K],
                rec[:, h : h + 1],
            )
        nc.sync.dma_start(probs_d.ap(), probs16[:, :])

        psum2 = ctx.enter_context(
            tc.tile_pool(name="psum2", bufs=3, space="PSUM")
        )
        psumt = ctx.enter_context(
            tc.tile_pool(name="psumt", bufs=2, space="PSUM")
        )

        # ---- phase 1.5: transpose probs16 -> probsT [128, 16, 64] ----
        for c in range(16):
            tp = psumt.tile([128, B], _F16, tag="tp")
            nc.tensor.transpose(
                tp[:, :], probs16[:, 128 * c : 128 * (c + 1)], id64[:, :]
            )
            nc.scalar.activation(
                probsT[:, c, :], tp[:, :], mybir.ActivationFunctionType.Copy
            )

        # ---- phase 2: staircase one-hot gather matmuls + evac + store ----
        for t in range(NBLK):
            prow = 32 * (t % 4)
            chunk = t // 4
            ot = opool.tile([128, NP], _F8, tag="ot")
            for j in range(4):
                w = 512 if j < 3 else NP - 3 * 512
                pst = psum2.tile([128, 512], _F32, tag="ps")
                for strm in range(PPC):
                    nc.tensor.matmul(
                        pst[B * strm : B * strm + B, 0:w],
                        probsT[prow : prow + 32, 8 * strm + chunk, :],
                        sel[
                            prow : prow + 32,
                            8 * strm + chunk,
                            512 * j : 512 * j + w,
                        ],
                        start=True,
                        stop=True,
                        tile_position=(prow, B * strm),
                    )
                # evac with x256 scale so fp8 e4m3 covers the prob range
                if (t * 4 + j) % 2 == 0:
                    nc.vector.tensor_scalar_mul(
                        ot[:, 512 * j : 512 * j + w], pst[:, 0:w], 256.0
                    )
                else:
                    nc.scalar.activation(
                        ot[:, 512 * j : 512 * j + w],
                        pst[:, 0:w],
                        mybir.ActivationFunctionType.Copy,
                        scale=256.0,
                    )
            nc.sync.dma_start(
                out_d.ap()
                .rearrange("s b np -> (s b) np")[:, NP * t : NP * (t + 1)],
                ot[:, :],
            )

    nc.compile()
    return nc


def _host_inputs(x, W, b, part):
    """Per-core inputs + per-(core,stream) position maps for host unpermute."""
    xT = np.ascontiguousarray(x.T.astype(np.float16))
    id64 = np.eye(B, dtype=np.float16)
    part = np.asarray(part).astype(np.int64, copy=False)
    in_maps = []
    pos_maps = []       # [core][strm] -> int32 [C] padded position or -1
    overflows = []      # [core][strm] -> list of (class, k) fallen out of NP
    for i in range(NCORES):
        r0 = NLOC * i
        wtb = np.empty((D + 1, NLOC), np.float16)
        wtb[:D] = W[r0 : r0 + NLOC].T
        wtb[D] = b[r0 : r0 + NLOC]

        sel = np.zeros((128, 16, NP), _F8NP)
        pm_core = []
        ov_core = []
        for s in range(PPC):
            kloc = part[PPC * i + s] - (PPC * i + s) * K  # [C] in [0, K)
            order = np.argsort(kloc, kind="stable")
            k_sorted = kloc[order]
            blk = k_sorted >> 5                            # 32-row block id
            # rank within block
            starts = np.searchsorted(blk, np.arange(NBLK))
            rank = np.arange(C, dtype=np.int64) - starts[blk]
            ok = rank < NP
            prow = 32 * (blk % 4) + (k_sorted & 31)
            dim1 = 8 * s + (blk >> 2)
            sel[prow[ok], dim1[ok], rank[ok]] = 1.0
            pos = np.where(ok, blk * NP + rank, -1).astype(np.int64)
            pm = np.empty(C, np.int64)
            pm[order] = pos
            pm_core.append(pm)
            if not ok.all():
                bad = order[~ok]
                ov_core.append([(int(c), int(kloc[c])) for c in bad])
            else:
                ov_core.append([])
        pos_maps.append(pm_core)
        overflows.append(ov_core)
        in_maps.append({"xT": xT, "wtb": wtb, "sel": sel, "id64": id64})
    return in_maps, pos_maps, overflows


def kernel(**inputs):
    global LAST_RESULTS
    x = np.asarray(inputs["input"], dtype=np.float32)
    W = np.asarray(inputs["W"], dtype=np.float32)
    b = np.asarray(inputs["b"], dtype=np.float32)
    part = np.asarray(inputs["partitionings"])
    assert x.shape == (B, D) and W.shape == (P * K, D)

    if "nc" not in _CACHE:
        _CACHE["nc"] = _build_nc()
    nc = _CACHE["nc"]

    in_maps, pos_maps, overflows = _host_inputs(x, W, b, part)
    trace = bool(int(os.environ.get("BASSK_TRACE", "0")))
    res = bass_utils.run_bass_kernel_spmd(
        nc,
        in_maps,
        core_ids=list(range(NCORES)),
        trace=trace,
        tmpdir=os.environ.get("BASSK_TRACE_DIR") or None,
    )
    LAST_RESULTS = res

    acc = np.zeros((B, C), np.float32)
    for i in range(NCORES):
        pout = res.results[i]["pout"]          # [PPC, B, NPAD] fp8 (x256)
        for s in range(PPC):
            acc += pout[s][:, pos_maps[i][s]].astype(np.float32) * (1 / 256.0)
            if overflows[i][s]:
                pr = res.results[i]["probs16"]  # [B, 2*KP] fp16
                for c, k in overflows[i][s]:
                    acc[:, c] += pr[:, KP * s + k].astype(np.float32)
    tot = acc.sum(axis=1, keepdims=True)
    return np.log(acc / tot + ESP).astype(np.float32)


# revision 17
# speedup vs baseline: 14.0317x; 1.3641x over previous
"""Trainium2 Bass kernel for nn_CombinatorialClassifier (segment_reduce).

Strategy (8 NeuronCores, tensor-parallel over the num_partitionings axis):
  Core i owns partitionings {2i, 2i+1} (a [2000, 2048] slice of W).
  On device:
    1. logits = x @ Wshard.T + b  (PE, fp16)
    2. per-partitioning softmax -> probs [64, 2048] fp16 (padded 1024/stream)
    3. probs transposed via PE -> probsT [128, 16, 64] (row k at partition
       k%128, chunk k//128; stream s rows at 1024s+k)
    4. "gather" as staircase one-hot matmuls: host sorts classes of each
       partitioning by partition idx k; each 32-row block of probsT is
       multiplied with a one-hot fp8 block S [32, NP] whose column j selects
       row k of the block -> PSUM [64, NP] = probs gathered in sorted order.
       Streams a/b go to PSUM partitions 0-63 / 64-127 (PE column tiling),
       evacuated to fp16 and DMA'd to DRAM in padded sorted order.
  Host: un-permute the 16 padded sorted streams, sum, normalize, log.

The Q7 ap_gather of the previous version (27 ns/idx, 1.43 ms total) is
replaced by PE matmuls at ~0.5 ns/gathered element.
"""

import os
from contextlib import ExitStack

import numpy as np

import concourse.bacc as bacc
import concourse.mybir as mybir
import concourse.tile as tile
from concourse import bass_utils

B, P, K, C, D = 64, 16, 1000, 50000, 2048
ESP = 1e-20
NCORES = 8
PPC = P // NCORES        # partitionings per core (2)
NLOC = PPC * K           # local logits width (2000)
NT = 500                 # logits matmul N-tile (PSUM bank: 500 fp32 <= 512)
NNT = NLOC // NT         # 4 N-tiles
DCH = D // 128           # 16 contraction chunks of 128
KP = 1024                # padded rows per stream (k space)
NBLK = KP // 32          # 32-row blocks per stream
NP = 1792                # padded sorted-class columns per block
NPAD = NBLK * NP         # padded sorted stream length (57344)

_F32 = mybir.dt.float32
_F16 = mybir.dt.float16
_F8 = mybir.dt.float8e4
_F8NP = mybir.dt.np(_F8)

_CACHE = {}
LAST_RESULTS = None


def _build_nc():
    nc = bacc.Bacc(
        "TRN2",
        target_bir_lowering=False,
        debug=False,
        enable_asserts=False,
        num_devices=NCORES,
    )
    xT_d = nc.dram_tensor("xT", [D, B], _F16, kind="ExternalInput")
    w8_d = nc.dram_tensor("w8", [D, NLOC], _F8, kind="ExternalInput")
    bias_d = nc.dram_tensor("bias", [1, NLOC], _F16, kind="ExternalInput")
    s_d = nc.dram_tensor("sel", [128, 16, NP], _F8, kind="ExternalInput")
    id_d = nc.dram_tensor("id64", [B, B], _F16, kind="ExternalInput")
    out_d = nc.dram_tensor("pout", [PPC, B, NPAD], _F8, kind="ExternalOutput")
    probs_d = nc.dram_tensor("probs16", [B, 2 * KP], _F16, kind="ExternalOutput")

    with tile.TileContext(nc) as tc, ExitStack() as ctx:
        const = ctx.enter_context(tc.tile_pool(name="const", bufs=1))
        wpool = ctx.enter_context(tc.tile_pool(name="w", bufs=3))
        spool = ctx.enter_context(tc.tile_pool(name="stats", bufs=1))
        opool = ctx.enter_context(tc.tile_pool(name="o", bufs=3))

        xt = const.tile([128, DCH, B], _F16)
        nc.sync.dma_start(xt[:], xT_d.ap().rearrange("(c p) b -> p c b", p=128))
        ones = const.tile([1, B], _F16)
        nc.vector.memset(ones[:], 1.0)
        bias = const.tile([1, NLOC], _F16)
        nc.sync.dma_start(bias[:], bias_d.ap())
        id64 = const.tile([B, B], _F16)
        nc.sync.dma_start(id64[:], id_d.ap())
        sel = const.tile([128, 16, NP], _F8)

        gsrc = const.tile([B, 2 * KP], _F32)
        probs16 = const.tile([B, 2 * KP], _F16)
        nc.vector.memset(probs16[:], 0.0)
        probsT = const.tile([128, 16, B], _F16)

        # ---- phase 1: logits = x @ Wshard.T + b, softmax -> probs16 ----
        with tc.tile_pool(name="psum1", bufs=1, space="PSUM") as psum1:
            ps = [
                psum1.tile([B, NT], _F32, tag=f"ps{n}", name=f"ps{n}")
                for n in range(NNT)
            ]
            for j in range(DCH):
                wt = wpool.tile([128, NLOC], _F8)
                # split the chunk into two DMAs for more engine parallelism
                nc.sync.dma_start(
                    wt[:, 0:K], w8_d[128 * j : 128 * (j + 1), 0:K]
                )
                nc.sync.dma_start(
                    wt[:, K:NLOC], w8_d[128 * j : 128 * (j + 1), K:NLOC]
                )
                for n in range(NNT):
                    nc.tensor.matmul(
                        ps[n][:],
                        xt[:, j, :],
                        wt[:, NT * n : NT * (n + 1)],
                        start=(j == 0),
                        stop=False,
                    )
            # sel is only needed in phase 2 — issue its DMA after the W
            # chunks so it doesn't delay the logits pipeline
            nc.sync.dma_start(sel[:], s_d.ap())
            for n in range(NNT):
                nc.tensor.matmul(
                    ps[n][:],
                    ones[:],
                    bias[:, NT * n : NT * (n + 1)],
                    start=False,
                    stop=True,
                )

            mx = spool.tile([B, NNT], _F32)
            for n in range(NNT):
                nc.vector.reduce_max(
                    mx[:, n : n + 1], ps[n][:], axis=mybir.AxisListType.X
                )
            neg = spool.tile([B, PPC], _F32)
            for h in range(PPC):
                nc.vector.tensor_tensor(
                    neg[:, h : h + 1],
                    mx[:, 2 * h : 2 * h + 1],
                    mx[:, 2 * h + 1 : 2 * h + 2],
                    op=mybir.AluOpType.max,
                )
            nc.vector.tensor_scalar_mul(neg[:], neg[:], -1.0)
            sacc = spool.tile([B, NNT], _F32)
            # bank n holds logits cols [500n, 500n+500) of the k-contiguous
            # [0, 2000) space; padded target col = 1024h + k
            goff = [0, 500, KP, KP + 500]
            for n in range(NNT):
                h = n // 2
                nc.scalar.activation(
                    gsrc[0:B, goff[n] : goff[n] + NT],
                    ps[n][:],
                    mybir.ActivationFunctionType.Exp,
                    bias=neg[:, h : h + 1],
                    accum_out=sacc[:, n : n + 1],
                )
        rec = spool.tile([B, PPC], _F32)
        for h in range(PPC):
            nc.vector.tensor_tensor(
                rec[:, h : h + 1],
                sacc[:, 2 * h : 2 * h + 1],
                sacc[:, 2 * h + 1 : 2 * h + 2],
                op=mybir.AluOpType.add,
            )
        nc.vector.reciprocal(rec[:], rec[:])
        for h in range(PPC):
            nc.vector.tensor_scalar_mul(
                probs16[0:B, KP * h : KP * h + K],
                gsrc[0:B, KP * h : KP * h + K],
                rec[:, h : h + 1],
            )
        nc.sync.dma_start(probs_d.ap(), probs16[:, :])

        psum2 = ctx.enter_context(
            tc.tile_pool(name="psum2", bufs=3, space="PSUM")
        )
        psumt = ctx.enter_context(
            tc.tile_pool(name="psumt", bufs=2, space="PSUM")
        )

        # ---- phase 1.5: transpose probs16 -> probsT [128, 16, 64] ----
        for c in range(16):
            tp = psumt.tile([128, B], _F16, tag="tp")
            nc.tensor.transpose(
                tp[:, :], probs16[:, 128 * c : 128 * (c + 1)], id64[:, :]
            )
            nc.scalar.activation(
                probsT[:, c, :], tp[:, :], mybir.ActivationFunctionType.Copy
            )

        # ---- phase 2: staircase one-hot gather matmuls + evac + store ----
        # two t-blocks interleaved -> 4 PE sub-tiles (2 rows x 2 cols) active
        for tt in range(0, NBLK, 2):
            ots = [
                opool.tile([128, NP], _F8, tag=f"ot{u}", name=f"ot{u}_{tt}")
                for u in range(2)
            ]
            for j in range(4):
                w = 512 if j < 3 else NP - 3 * 512
                for u in range(2):
                    t = tt + u
                    prow = 32 * (t % 4)
                    chunk = t // 4
                    pst = psum2.tile([128, 512], _F32, tag=f"ps{u}")
                    for strm in range(PPC):
                        nc.tensor.matmul(
                            pst[B * strm : B * strm + B, 0:w],
                            probsT[prow : prow + 32, 8 * strm + chunk, :],
                            sel[
                                prow : prow + 32,
                                8 * strm + chunk,
                                512 * j : 512 * j + w,
                            ],
                            start=True,
                            stop=True,
                            tile_position=(prow, B * strm),
                        )
                    # evac with x256 scale so fp8 e4m3 covers the prob range
                    if (t * 4 + j) % 2 == 0:
                        nc.vector.tensor_scalar_mul(
                            ots[u][:, 512 * j : 512 * j + w], pst[:, 0:w], 256.0
                        )
                    else:
                        nc.scalar.activation(
                            ots[u][:, 512 * j : 512 * j + w],
                            pst[:, 0:w],
                            mybir.ActivationFunctionType.Copy,
                            scale=256.0,
                        )
            for u in range(2):
                t = tt + u
                nc.sync.dma_start(
                    out_d.ap()
                    .rearrange("s b np -> (s b) np")[:, NP * t : NP * (t + 1)],
                    ots[u][:, :],
                )

    nc.compile()
    return nc


def _host_inputs(x, W, b, part):
    """Per-core inputs + per-(core,stream) position maps for host unpermute."""
    xT = np.ascontiguousarray(x.T.astype(np.float16))
    id64 = np.eye(B, dtype=np.float16)
    part = np.asarray(part).astype(np.int64, copy=False)
    in_maps = []
    pos_maps = []       # [core][strm] -> int32 [C] padded position or -1
    overflows = []      # [core][strm] -> list of (class, k) fallen out of NP
    for i in range(NCORES):
        r0 = NLOC * i
        w8 = W[r0 : r0 + NLOC].T.astype(_F8NP)
        bias = b[r0 : r0 + NLOC].astype(np.float16)[None, :]

        sel = np.zeros((128, 16, NP), _F8NP)
        pm_core = []
        ov_core = []
        for s in range(PPC):
            kloc = part[PPC * i + s] - (PPC * i + s) * K  # [C] in [0, K)
            order = np.argsort(kloc, kind="stable")
            k_sorted = kloc[order]
            blk = k_sorted >> 5                            # 32-row block id
            # rank within block
            starts = np.searchsorted(blk, np.arange(NBLK))
            rank = np.arange(C, dtype=np.int64) - starts[blk]
            ok = rank < NP
            prow = 32 * (blk % 4) + (k_sorted & 31)
            dim1 = 8 * s + (blk >> 2)
            sel[prow[ok], dim1[ok], rank[ok]] = 1.0
            pos = np.where(ok, blk * NP + rank, -1).astype(np.int64)
            pm = np.empty(C, np.int64)
            pm[order] = pos
            pm_core.append(pm)
            if not ok.all():
                bad = order[~ok]
                ov_core.append([(int(c), int(kloc[c])) for c in bad])
            else:
                ov_core.append([])
        pos_maps.append(pm_core)
        overflows.append(ov_core)
        in_maps.append(
            {"xT": xT, "w8": w8, "bias": bias, "sel": sel, "id64": id64}
        )
    return in_maps, pos_maps, overflows


def kernel(**inputs):
    global LAST_RESULTS
    x = np.asarray(inputs["input"], dtype=np.float32)
    W = np.asarray(inputs["W"], dtype=np.float32)
    b = np.asarray(inputs["b"], dtype=np.float32)
    part = np.asarray(inputs["partitionings"])
    assert x.shape == (B, D) and W.shape == (P * K, D)

    if "nc" not in _CACHE:
        _CACHE["nc"] = _build_nc()
    nc = _CACHE["nc"]

    in_maps, pos_maps, overflows = _host_inputs(x, W, b, part)
    trace = bool(int(os.environ.get("BASSK_TRACE", "0")))
    res = bass_utils.run_bass_kernel_spmd(
        nc,
        in_maps,
        core_ids=list(range(NCORES)),
        trace=trace,
        tmpdir=os.environ.get("BASSK_TRACE_DIR") or None,
    )
    LAST_RESULTS = res

    acc = np.zeros((B, C), np.float32)
    for i in range(NCORES):
        pout = res.results[i]["pout"]          # [PPC, B, NPAD] fp8 (x256)
        for s in range(PPC):
            acc += pout[s][:, pos_maps[i][s]].astype(np.float32) * (1 / 256.0)
            if overflows[i][s]:
                pr = res.results[i]["probs16"]  # [B, 2*KP] fp16
                for c, k in overflows[i][s]:
                    acc[:, c] += pr[:, KP * s + k].astype(np.float32)
    tot = acc.sum(axis=1, keepdims=True)
    return np.log(acc / tot + ESP).astype(np.float32)
